# revision 1
# baseline (speedup 1.0000x reference)
"""Multi-head attention (B=8, N=1024, C=768, H=12) on 8 Trainium2 NeuronCores.

Strategy: pure data parallelism over the batch dimension — each of the 8
cores computes full attention for one batch element; weights are
replicated. No collectives needed.

Per-core dataflow (all matmuls expressed as out = lhsT.T @ rhs on the PE):
  1. x loaded in 8 per-token-chunk casting DMAs (bf16); a short warm-up
     matmul stream raises the PE p-state, then xT is built with PE
     identity-transposes pipelined behind the DMAs (4 PSUM slots).
  2. qkT = w_qkv[:, :1536].T @ xT  (q,k feature-major, bf16)
     v    = x @ w_qkv[:, 1536:]    (v token-major, bf16, 96-col head slots
                                    with ones at col 64 -> softmax denom)
  3. per head pair (2 heads share a 128-row qkT chunk -> row-tiled K=64):
       scoresT[m,n] = k_h @ q_h^T      (lhsT = kT slice, rhs = qT slice)
       expT = exp(scale * scoresT)     (ScalarE; max-subtraction skipped:
                                        |scores*scale| < ~2, exp safe)
       U^T[d,n] += v_aug[m,d] expT[m,n]  (v stationary, 128-wide for FWL;
                                          row 64 = softmax denominator)
       U^T -> token-major via one DMA-XBAR transpose per head; normalize
       with a single broadcast tensor_tensor multiply (recip per token).
  4. aoT via PE transposes per pair; y = aoT.T @ w_proj + b
All matmul operands bf16; accumulation fp32 in PSUM.
"""

import os
import sys

for _p in ("/opt/trn_rl_repo", "/root/.axon_site/_ro/trn_rl_repo"):
    if os.path.isdir(_p) and _p not in sys.path:
        sys.path.append(_p)

from contextlib import ExitStack

import numpy as np

import concourse.bass as bass
import concourse.tile as tile
from concourse import bacc, mybir
from concourse.bass_utils import run_bass_kernel_spmd
from concourse.masks import make_identity

FP = mybir.dt.float32
BF16 = mybir.dt.bfloat16
N_CORES = 8
T = 1024  # tokens per core (batch element)
C = 768
H = 12
D = 64
SCALE = D ** (-0.5)
TC = T // 128  # 8 token chunks
CCH = C // 128  # 6 channel chunks
NPAIR = H // 2  # 6 head pairs
VS = 96  # v columns per head slot (64 data + ones/pad)

Exp = mybir.ActivationFunctionType.Exp
Mult = mybir.AluOpType.mult


def build(n_cores: int = N_CORES):
    nc = bacc.Bacc(
        "TRN2", target_bir_lowering=False, debug=False, num_devices=n_cores
    )
    wdma = nc.gpsimd.dma_start
    x = nc.declare_dram_parameter("x", [T, C], FP, isOutput=False)
    w_qkv = nc.declare_dram_parameter("w_qkv", [C, 3 * C], FP, isOutput=False)
    w_proj = nc.declare_dram_parameter("w_proj", [C, C], FP, isOutput=False)
    b_proj = nc.declare_dram_parameter("b_proj", [C], FP, isOutput=False)
    out = nc.declare_dram_parameter("out", [T, C], FP, isOutput=True)

    xa, wqa, wpa, outa = x.ap(), w_qkv.ap(), w_proj.ap(), out.ap()
    ba = b_proj.ap()
    b_bcast_src = bass.AP(tensor=ba.tensor, offset=ba.offset, ap=[[0, 128]] + ba.ap)

    with tile.TileContext(nc) as tc, ExitStack() as ctx:
        consts = ctx.enter_context(tc.tile_pool(name="consts", bufs=1))
        xs_pool = ctx.enter_context(tc.tile_pool(name="xstage", bufs=4))
        xT_pool = ctx.enter_context(tc.tile_pool(name="xT", bufs=1))
        wq1_pool = ctx.enter_context(tc.tile_pool(name="wq1", bufs=1))
        wq2_pool = ctx.enter_context(tc.tile_pool(name="wq2", bufs=1))
        wp_pool = ctx.enter_context(tc.tile_pool(name="wp", bufs=1))
        qk_pool = ctx.enter_context(tc.tile_pool(name="qk", bufs=12))
        v_pool = ctx.enter_context(tc.tile_pool(name="v65", bufs=TC))
        exp_pool = ctx.enter_context(tc.tile_pool(name="expT", bufs=4))
        uT_pool = ctx.enter_context(tc.tile_pool(name="uT", bufs=2))
        at_pool = ctx.enter_context(tc.tile_pool(name="atmp", bufs=2))
        r_pool = ctx.enter_context(tc.tile_pool(name="r", bufs=2))
        ao_pool = ctx.enter_context(tc.tile_pool(name="ao", bufs=1))
        aoT_pool = ctx.enter_context(tc.tile_pool(name="aoT", bufs=1))
        y_pool = ctx.enter_context(tc.tile_pool(name="y", bufs=2))
        # PSUM: accA 2x1 + accB 2x1 + sc 2x2 = 8 banks
        accA = ctx.enter_context(tc.tile_pool(name="accA", bufs=2, space="PSUM"))
        accB = ctx.enter_context(tc.tile_pool(name="accB", bufs=2, space="PSUM"))
        sc_psum = ctx.enter_context(tc.tile_pool(name="sc", bufs=2, space="PSUM"))

        identity_h = consts.tile([128, 128], BF16)
        make_identity(nc, identity_h)

        # ---- PE warm-up: back-to-back matmuls raise the p-state while the
        # first input DMAs land, so real work starts at full clock.
        warm = consts.tile([128, 128], BF16)
        nc.vector.memset(warm[:], 0.0)
        for i in range(28):
            wps = accA.tile([128, 512], FP, tag="accA", name="accA")
            nc.tensor.matmul(
                wps[:, 0:128], warm[:], warm[:, 0:128], start=True, stop=True
            )

        # ---- input DMAs (gpsimd SWDGE does fp32->bf16 casts); emission
        # order is the queue order, so earliest-needed data goes first.
        def grouped(src_ap, width, ngrp, col0):
            row_step = src_ap.ap[0][0]
            return bass.AP(
                tensor=src_ap.tensor,
                offset=src_ap.offset + col0,
                ap=[[row_step, 128], [128 * row_step, ngrp], [1, width]],
            )

        xs = [xs_pool.tile([128, 2, C], BF16, tag="xs", name="xs") for _ in range(4)]

        def dma_x(i):
            # one token chunk per DMA: the first transposes start as soon as
            # chunk 0 lands instead of waiting for a 2-chunk transfer
            for k in range(2):
                wdma(
                    xs[i][:, k, :],
                    bass.AP(
                        tensor=xa.tensor,
                        offset=xa.offset + (2 * i + k) * 128 * C,
                        ap=[[C, 128], [1, C]],
                    ),
                )

        wq1_all = wq1_pool.tile([128, CCH, 2 * C], BF16, tag="wq1", name="wq1")

        def dma_wq1(j):
            lo = j * 128
            wdma(wq1_all[:, :, lo : lo + 128], grouped(wqa, 128, CCH, lo))

        wq2_all = wq2_pool.tile([128, CCH, C], BF16, tag="wq2", name="wq2")
        wp_all = wp_pool.tile([128, CCH, C], BF16, tag="wp", name="wp")
        b_bcast = consts.tile([128, C], FP)

        dma_x(0)
        dma_x(1)
        dma_x(2)
        dma_x(3)
        dma_wq1(0)
        dma_wq1(6)
        dma_wq1(1)
        dma_wq1(7)
        wdma(wq2_all[:], grouped(wqa, C, CCH, 2 * C))
        dma_wq1(2)
        dma_wq1(8)
        wdma(wp_all[:], grouped(wpa, C, CCH, 0))
        dma_wq1(3)
        dma_wq1(9)
        dma_wq1(4)
        dma_wq1(10)
        dma_wq1(5)
        dma_wq1(11)
        nc.sync.dma_start(b_bcast[:], b_bcast_src)

        xT_all = xT_pool.tile([128, CCH, T], BF16, tag="xT", name="xT")

        def emit_xpose(t):
            # PE identity-transpose [128 tok, 128 ch] -> xT_all[:, c, t-slice]
            # 4 PSUM slots (accA+accB) and alternating evac engines keep the
            # transpose stream dense behind the x DMAs.
            for c in range(CCH):
                k = (t * CCH + c) % 4
                pool, tg = ((accA, "accA"), (accB, "accB"))[k // 2]
                ps = pool.tile([128, 512], FP, tag=tg, name=tg)
                psh = ps[:, 0:256].bitcast(BF16)
                nc.tensor.transpose(
                    psh[:, 0:128],
                    xs[t // 2][:, t % 2, c * 128 : (c + 1) * 128],
                    identity_h[:],
                )
                if (t * CCH + c) % 2 == 0:
                    nc.vector.tensor_copy(
                        xT_all[:, c, t * 128 : (t + 1) * 128], psh[:, 0:128]
                    )
                else:
                    nc.scalar.copy(
                        xT_all[:, c, t * 128 : (t + 1) * 128], psh[:, 0:128]
                    )

        for t in range(TC):
            emit_xpose(t)

        qkT = {}

        def emit_f1(j):
            # qkT[j] = w_qkv[:, j-chunk].T @ x^T
            qkT[j] = qk_pool.tile([128, T], BF16, tag="qk", name="qk")
            for nh in range(2):
                ps = accA.tile([128, 512], FP, tag="accA", name="accA")
                for c in range(CCH):
                    nc.tensor.matmul(
                        ps[:],
                        wq1_all[:, c, j * 128 : (j + 1) * 128],
                        xT_all[:, c, nh * 512 : (nh + 1) * 512],
                        start=(c == 0),
                        stop=(c == CCH - 1),
                    )
                nc.vector.tensor_copy(qkT[j][:, nh * 512 : (nh + 1) * 512], ps[:])

        v65 = [
            v_pool.tile([128, 13 * VS], BF16, tag="v65", name="v65")
            for _ in range(TC)
        ]

        def emit_f2(t):
            # v[t] = x[t-chunk] @ w_qkv[:, v-cols]; ones at col 64 of each
            # 96-col head slot (-> denominator rows); pad group 12 covered.
            vt = v65[t]
            nc.vector.memset(
                vt[:].rearrange("p (g d) -> p g d", d=VS)[:, :, D:], 1.0
            )
            nc.vector.memset(vt[:, 12 * VS : 12 * VS + D], 1.0)
            for nh in range(2):
                ps = accA.tile([128, 512], FP, tag="accA", name="accA")
                for c in range(CCH):
                    nc.tensor.matmul(
                        ps[:, 0:384],
                        xT_all[:, c, t * 128 : (t + 1) * 128],
                        wq2_all[:, c, nh * 384 : (nh + 1) * 384],
                        start=(c == 0),
                        stop=(c == CCH - 1),
                    )
                nc.vector.tensor_copy(
                    vt[:, nh * 6 * VS : (nh + 1) * 6 * VS].rearrange(
                        "p (g d) -> p g d", d=VS
                    )[:, :, 0:D],
                    ps[:, 0:384].rearrange("p (g d) -> p g d", g=6),
                )

        def emit_scores_exp(p, eAB):
            for j in range(TC):  # key-token chunks (m)
                psAB = [
                    sc_psum.tile([128, T], FP, tag="sc", name="sc")
                    for _ in range(2)
                ]
                # alternate the two row-tiles (T0/T8) every matmul so each
                # tile's LDWEIGHTS overlaps the other tile's matmul
                for nh in range(2):
                    for half in range(2):
                        base = 64 * half
                        nc.tensor.matmul(
                            psAB[half][:, nh * 512 : (nh + 1) * 512],
                            qkT[6 + p][base : base + 64, j * 128 : (j + 1) * 128],
                            qkT[p][base : base + 64, nh * 512 : (nh + 1) * 512],
                            start=True,
                            stop=True,
                        )
                for half in range(2):
                    nc.scalar.activation(
                        eAB[half][:, j, :], psAB[half][:], Exp, scale=SCALE
                    )

        ao_all = ao_pool.tile([128, TC, C], BF16, tag="ao", name="ao")

        def emit_u(p, eAB, last=False):
            # U^T[d, n] = sum_m v_aug[m, d] expT[m, n]; v stationary
            # (128-wide slice for FWL), expT moving at N=512.
            for half in range(2):
                h = 2 * p + half
                e = eAB[half]
                # for the final pair, half 1's normalize chain is routed via
                # ScalarE (exp stream is done) so both halves run in parallel
                alt = last and half == 1
                ups = [
                    accB.tile([128, 512], FP, tag="accB", name="accB")
                    for _ in range(2)
                ]
                for j in range(TC):
                    for nh in range(2):
                        nc.tensor.matmul(
                            ups[nh][:],
                            v65[j][:, h * VS : h * VS + 128],
                            e[:, j, nh * 512 : (nh + 1) * 512],
                            start=(j == 0),
                            stop=(j == TC - 1),
                        )
                uT_sb = uT_pool.tile([80, T], BF16, tag="uT", name="uT")
                for nh in range(2):
                    if alt:
                        nc.scalar.copy(
                            uT_sb[:, nh * 512 : (nh + 1) * 512], ups[nh][0:80, :]
                        )
                    else:
                        nc.vector.tensor_copy(
                            uT_sb[:, nh * 512 : (nh + 1) * 512], ups[nh][0:80, :]
                        )
                # token-major via DMA-XBAR: atmp[:, g, k] = uT_sb[k, g*128+p]
                atmp = at_pool.tile([128, TC, 80], BF16, tag="atmp", name="atmp")
                (nc.scalar if alt else nc.sync).dma_start_transpose(
                    atmp[:], uT_sb[:]
                )
                r = r_pool.tile([128, TC], FP, tag="r", name="r")
                nc.vector.reciprocal(r[:], atmp[:, :, D])
                rap = r[:]
                rb = bass.AP(
                    tensor=rap.tensor,
                    offset=rap.offset,
                    ap=[rap.ap[0], rap.ap[1], [0, D]],
                )
                nc.vector.tensor_tensor(
                    ao_all[:, :, h * D : (h + 1) * D],
                    atmp[:, :, 0:D],
                    rb,
                    op=Mult,
                )

        aoT_all = aoT_pool.tile([128, CCH, T], BF16, tag="aoT", name="aoT")

        def emit_aotp(c, pool=None, tg="accA"):
            # pair c filled ao cols c*128:(c+1)*128 == proj lhsT chunk c
            pool = pool or accA
            for t in range(TC):
                ps = pool.tile([128, 512], FP, tag=tg, name=tg)
                psh = ps[:, 0:256].bitcast(BF16)
                nc.tensor.transpose(
                    psh[:, 0:128],
                    ao_all[:, t, c * 128 : (c + 1) * 128],
                    identity_h[:],
                )
                nc.vector.tensor_copy(
                    aoT_all[:, c, t * 128 : (t + 1) * 128], psh[:, 0:128]
                )

        # proj chains for the first two token chunks open early (c 0-4) on
        # accA+sc while pair 5's normalize chain runs — filling the ~5 us
        # the PE would otherwise idle before aotp(5) (which uses accB).
        open_ps = {}

        def emit_proj_open():
            for t in range(2):
                for nh in range(2):
                    pool, tg = ((accA, "accA"), (sc_psum, "sc"))[t]
                    ps = pool.tile([128, 512], FP, tag=tg, name=tg)
                    open_ps[(t, nh)] = ps
                    for c in range(CCH - 1):
                        nc.tensor.matmul(
                            ps[:, 0:384],
                            aoT_all[:, c, t * 128 : (t + 1) * 128],
                            wp_all[:, c, nh * 384 : (nh + 1) * 384],
                            start=(c == 0),
                            stop=False,
                        )

        def emit_proj_close():
            for t in range(2):
                y = y_pool.tile([128, C], FP, tag="y", name="y")
                for nh in range(2):
                    ps = open_ps[(t, nh)]
                    nc.tensor.matmul(
                        ps[:, 0:384],
                        aoT_all[:, CCH - 1, t * 128 : (t + 1) * 128],
                        wp_all[:, CCH - 1, nh * 384 : (nh + 1) * 384],
                        start=False,
                        stop=True,
                    )
                    nc.vector.tensor_add(
                        y[:, nh * 384 : (nh + 1) * 384],
                        ps[:, 0:384],
                        b_bcast[:, nh * 384 : (nh + 1) * 384],
                    )
                    eng = nc.scalar if (2 * t + nh) % 2 == 0 else nc.sync
                    eng.dma_start(
                        bass.AP(
                            tensor=outa.tensor,
                            offset=outa.offset + t * 128 * C + nh * 384,
                            ap=[[C, 128], [1, 384]],
                        ),
                        y[:, nh * 384 : (nh + 1) * 384],
                    )

        def emit_proj():
            for t in range(2, TC):
                y = y_pool.tile([128, C], FP, tag="y", name="y")
                for nh in range(2):
                    k3 = (2 * t + nh) % 3
                    pool = (accA, accB, sc_psum)[k3]
                    tg = ("accA", "accB", "sc")[k3]
                    ps = pool.tile([128, 512], FP, tag=tg, name=tg)
                    for c in range(CCH):
                        nc.tensor.matmul(
                            ps[:, 0:384],
                            aoT_all[:, c, t * 128 : (t + 1) * 128],
                            wp_all[:, c, nh * 384 : (nh + 1) * 384],
                            start=(c == 0),
                            stop=(c == CCH - 1),
                        )
                    nc.vector.tensor_add(
                        y[:, nh * 384 : (nh + 1) * 384],
                        ps[:, 0:384],
                        b_bcast[:, nh * 384 : (nh + 1) * 384],
                    )
                    eng = nc.scalar if (2 * t + nh) % 2 == 0 else nc.sync
                    eng.dma_start(
                        bass.AP(
                            tensor=outa.tensor,
                            offset=outa.offset + t * 128 * C + nh * 384,
                            ap=[[C, 128], [1, 384]],
                        ),
                        y[:, nh * 384 : (nh + 1) * 384],
                    )

        # ---- woven emission schedule ----
        emit_f1(0)
        emit_f1(6)
        eAB_list = []

        def new_pair():
            eAB = [
                exp_pool.tile([128, TC, T], BF16, tag="expT", name="expT")
                for _ in range(2)
            ]
            eAB_list.append(eAB)
            return eAB

        emit_scores_exp(0, new_pair())
        for t in range(TC):
            emit_f2(t)
        emit_f1(1)
        emit_f1(7)
        emit_scores_exp(1, new_pair())
        for p in range(2, NPAIR):
            emit_f1(p)
            emit_f1(6 + p)
            emit_u(p - 2, eAB_list[p - 2])
            emit_aotp(p - 2)
            emit_scores_exp(p, new_pair())
        emit_u(NPAIR - 2, eAB_list[NPAIR - 2])
        emit_aotp(NPAIR - 2)
        emit_u(NPAIR - 1, eAB_list[NPAIR - 1], last=True)
        emit_proj_open()
        emit_aotp(NPAIR - 1, pool=accB, tg="accB")
        emit_proj_close()
        emit_proj()

    nc.finalize()
    return nc


_NC_CACHE = {}


def _get_nc():
    if "nc" not in _NC_CACHE:
        _NC_CACHE["nc"] = build()
    return _NC_CACHE["nc"]


def kernel(x, w_qkv, w_proj, b_proj):
    """Full inputs in, full output out. Shards batch across 8 NeuronCores."""
    assert x.shape == (N_CORES, T, C), x.shape
    nc = _get_nc()
    in_maps = [
        {
            "x": np.ascontiguousarray(x[i], dtype=np.float32),
            "w_qkv": np.ascontiguousarray(w_qkv, dtype=np.float32),
            "w_proj": np.ascontiguousarray(w_proj, dtype=np.float32),
            "b_proj": np.ascontiguousarray(b_proj, dtype=np.float32),
        }
        for i in range(N_CORES)
    ]
    res = run_bass_kernel_spmd(nc, in_maps, list(range(N_CORES)))
    return np.stack([res.results[i]["out"] for i in range(N_CORES)], axis=0)

